# revision 8
# baseline (speedup 1.0000x reference)
"""Trainium2 Bass kernel for nn_Node_Transformation.

Computes, for row n:
    out[n] = emb_weight[node_type[n]]                 if node_type[n] != item_id
             x[n] @ W.T + b                           if node_type[n] == item_id

Equivalent formulation used on device (exact, float-add commutative):
    table2       = emb_weight with row item_id replaced by b
    out[n]       = table2[node_type[n]] + mask[n] * (x[n] @ W.T)

Sharding: data-parallel over N across 8 NeuronCores. Weights/table replicated.
Per-core rows are laid out "partition-major": global (in-shard) row index
r = p*F + f  for partition p in [0,128) and tile column f in [0,F).
"""

import os
import numpy as np

import concourse.bass as bass
import concourse.bacc as bacc
import concourse.mybir as mybir
from concourse.tile import TileContext
from concourse.bass import IndirectOffsetOnAxis
from concourse.bass_utils import run_bass_kernel_spmd
from concourse.masks import make_identity

# ---- problem constants (hardcoded per contest contract) ----
N = 500000
IN_CH = 256
HID = 128
NUM_T = 8
NCORES = 8
P = 128
NSH = N // NCORES          # 62500 real rows per core
F = (NSH + P - 1) // P     # 489 tile columns
PAD = P * F                # 62592 padded rows per core

_CACHE = {}


def _ensure_axon_profile_hook():
    """bass_utils' trace path imports antenv.axon_hooks, which this image
    lacks. Register an equivalent module backed by the axon PJRT .so so
    trace=True (or BASS_TRACE=1) works instead of crashing."""
    try:
        import antenv.axon_hooks  # noqa: F401
        return
    except ImportError:
        pass
    import sys
    import types

    hook = None
    try:
        from trn_agent_boot.trn_boot import _ntff_profile_via_ctypes

        hook = _ntff_profile_via_ctypes("/opt/axon/libaxon_pjrt.so")
    except Exception:
        hook = None
    mod = types.ModuleType("antenv.axon_hooks")
    mod.get_axon_ntff_profile_hook = lambda: hook
    mod.set_axon_ntff_profile_hook = lambda h: None
    sys.modules["antenv.axon_hooks"] = mod
    try:
        import antenv

        antenv.axon_hooks = mod
    except ImportError:
        pass


def _build(item: int) -> bass.Bass:
    nc = bacc.Bacc("TRN2")
    f32 = mybir.dt.float32
    i32 = mybir.dt.int32

    x_d = nc.dram_tensor("x", [PAD, IN_CH], f32, kind="ExternalInput")
    nt_d = nc.dram_tensor("nt", [PAD], i32, kind="ExternalInput")
    t2_d = nc.dram_tensor("table2", [NUM_T, HID], f32, kind="ExternalInput")
    wt_d = nc.dram_tensor("wt", [IN_CH, HID], f32, kind="ExternalInput")
    out_d = nc.dram_tensor("out", [PAD, HID], f32, kind="ExternalOutput")

    x_v = x_d[:].rearrange("(p f) c -> p f c", p=P)     # [128, F, 256]
    nt_v = nt_d[:].rearrange("(p f) -> p f", p=P)       # [128, F]
    out_v = out_d[:].rearrange("(p f) h -> p f h", p=P) # [128, F, 128]

    with TileContext(nc) as tc:
        with (
            tc.tile_pool(name="singles", bufs=1) as singles,
            tc.tile_pool(name="xp", bufs=4) as xpool,
            tc.tile_pool(name="tp", bufs=4) as tpool,
            tc.tile_pool(name="op", bufs=4) as opool,
            tc.tile_pool(name="ps", bufs=2, space="PSUM") as pspool,
        ):
            ident = singles.tile([P, P], f32)
            make_identity(nc, ident)

            wt_s = singles.tile([P, 2, HID], f32)
            nc.sync.dma_start(out=wt_s[:], in_=wt_d[:].rearrange("(c k) h -> k c h", c=2))

            nt_all = singles.tile([P, F], i32)
            nc.sync.dma_start(out=nt_all[:], in_=nt_v)
            ntf = singles.tile([P, F], f32)
            nc.vector.tensor_copy(ntf[:], nt_all[:])
            eq_all = singles.tile([P, F], f32)
            nc.vector.tensor_scalar(
                out=eq_all[:], in0=ntf[:], scalar1=float(item), scalar2=None,
                op0=mybir.AluOpType.is_equal,
            )

            for f in range(F):
                x_t = xpool.tile([P, IN_CH], f32, tag="x")
                nc.sync.dma_start(out=x_t[:], in_=x_v[:, f, :])
                xm = xpool.tile([P, IN_CH], f32, tag="xm")
                nc.vector.tensor_tensor(
                    out=xm[:], in0=x_t[:],
                    in1=eq_all[:, f : f + 1].to_broadcast([P, IN_CH]),
                    op=mybir.AluOpType.mult,
                )

                pt = pspool.tile([P, P], f32, tag="pt")
                pt2 = pspool.tile([P, P], f32, tag="pt2")
                nc.tensor.transpose(pt[:], xm[:, 0:P], ident[:])
                nc.tensor.transpose(pt2[:], xm[:, P : 2 * P], ident[:])
                xt = tpool.tile([P, 2, P], f32, tag="xt")
                nc.vector.tensor_copy(xt[:, 0, :], pt[:])
                nc.vector.tensor_copy(xt[:, 1, :], pt2[:])

                lin = pspool.tile([P, HID], f32, tag="lin")
                nc.tensor.matmul(out=lin[:], lhsT=xt[:, 0, :], rhs=wt_s[:, 0, :],
                                 start=True, stop=False)
                nc.tensor.matmul(out=lin[:], lhsT=xt[:, 1, :], rhs=wt_s[:, 1, :],
                                 start=False, stop=True)

                emb_t = opool.tile([P, HID], f32, tag="emb")
                nc.gpsimd.indirect_dma_start(
                    out=emb_t[:], out_offset=None, in_=t2_d[:],
                    in_offset=IndirectOffsetOnAxis(ap=nt_all[:, f : f + 1], axis=0),
                )
                o_t = opool.tile([P, HID], f32, tag="o")
                nc.vector.tensor_tensor(out=o_t[:], in0=emb_t[:], in1=lin[:],
                                        op=mybir.AluOpType.add)
                nc.scalar.dma_start(out=out_v[:, f, :], in_=o_t[:])
    nc.compile()
    return nc


def _prepare(inputs):
    x = np.asarray(inputs["x"], dtype=np.float32)
    nt = np.asarray(inputs["node_type"]).astype(np.int32)
    item = int(np.asarray(inputs["item_id"]))
    emb = np.asarray(inputs["emb_weight"], dtype=np.float32)
    W = np.asarray(inputs["W"], dtype=np.float32)
    b = np.asarray(inputs["b"], dtype=np.float32)

    table2 = emb.copy()
    table2[item] = b
    wt = np.ascontiguousarray(W.T)  # [IN_CH, HID]
    pad_val = np.int32((item + 1) % NUM_T)  # never selected

    in_maps = []
    for c in range(NCORES):
        xp = np.zeros((PAD, IN_CH), np.float32)
        xp[:NSH] = x[c * NSH : (c + 1) * NSH]
        ntp = np.full(PAD, pad_val, np.int32)
        ntp[:NSH] = nt[c * NSH : (c + 1) * NSH]
        in_maps.append({"x": xp, "nt": ntp, "table2": table2, "wt": wt})
    return item, in_maps


def _run(inputs, trace=False):
    _ensure_axon_profile_hook()
    item, in_maps = _prepare(inputs)
    if item not in _CACHE:
        _CACHE[item] = _build(item)
    nc = _CACHE[item]
    res = run_bass_kernel_spmd(nc, in_maps, core_ids=list(range(NCORES)), trace=trace)
    out = np.empty((N, HID), np.float32)
    for c in range(NCORES):
        out[c * NSH : (c + 1) * NSH] = res.results[c]["out"][:NSH]
    return out, res


def kernel(**inputs) -> np.ndarray:
    out, _ = _run(inputs, trace=bool(os.environ.get("KERNEL_TRACE")))
    return out
